# revision 12
# baseline (speedup 1.0000x reference)
"""AttnBlock (GroupNorm + 1x1-conv QKV + single-head spatial attention + proj
+ residual) on 8 Trainium2 NeuronCores.

Sharding: pure data-parallel over batch — 16 samples / 8 cores = 2 samples per
core; weights broadcast. No collectives; gather on host.

Per-core kernel formulation (per sample, C=512 channels, N=1024 spatial):
  h   = groupnorm(x)                              (stats via PE indicator matmuls)
  q,k = qkv_w[:1024] @ h        (C on partitions, spatial free)
  vT  = h^T @ v_w^T             (spatial on partitions, C free)
  s   = k^T q                   (keys j on partitions, queries i free)
  e   = exp(s * C^-0.5)         (no max-subtraction: logits are O(1) by construction)
  S   = ones^T e                (softmax denominators via PE reduction)
  o   = vT^T e / S              (attention output, C on partitions)
  y   = x + proj_w @ o + proj_b
All matmuls run as float32r (full fp32 storage, single-pass PE mode).
"""

import numpy as np

import concourse.bass as bass
import concourse.tile as tile
from concourse import bacc, mybir
from concourse.bass_utils import run_bass_kernel_spmd

B, C, H, W = 16, 512, 32, 32
N = H * W              # 1024 spatial positions
G = 32                 # groups
GS = C // G            # 16 channels per group
NCORES = 8
SPC = B // NCORES      # samples per core
EPS = 1e-6
SCALE = float(C) ** -0.5
KT = C // 128          # 4 channel tiles of 128
NT = N // 128          # 8 spatial tiles of 128
NH = N // 512          # 2 free-dim halves of 512

F32 = mybir.dt.float32
F32R = mybir.dt.float32r

_BUILD_CACHE = {}
LAST_RESULT = None  # BassKernelResults of the most recent run (for test harness)


def _build():
    nc = bacc.Bacc("TRN2", target_bir_lowering=False, debug=False)

    x_ext = nc.declare_dram_parameter("x", [SPC, C, N], F32, isOutput=False)
    nw_ext = nc.declare_dram_parameter("norm_w", [C], F32, isOutput=False)
    nb_ext = nc.declare_dram_parameter("norm_b", [C], F32, isOutput=False)
    qkvwT_ext = nc.declare_dram_parameter("qkvwT", [C, 3 * C], F32R, isOutput=False)
    qkvb_ext = nc.declare_dram_parameter("qkv_b", [3 * C], F32, isOutput=False)
    projwT_ext = nc.declare_dram_parameter("projwT", [C, C], F32R, isOutput=False)
    projb_ext = nc.declare_dram_parameter("proj_b", [C], F32R, isOutput=False)
    ind_ext = nc.declare_dram_parameter("ind16", [128, 8], F32R, isOutput=False)
    indT_ext = nc.declare_dram_parameter("ind16T", [8, 128], F32R, isOutput=False)
    ones_ext = nc.declare_dram_parameter("ones", [N], F32R, isOutput=False)
    y_ext = nc.declare_dram_parameter("y", [SPC, C, N], F32, isOutput=True)

    Identity = mybir.ActivationFunctionType.Identity
    Exp = mybir.ActivationFunctionType.Exp
    Sqrt = mybir.ActivationFunctionType.Sqrt
    mult = mybir.AluOpType.mult
    add = mybir.AluOpType.add

    with tile.TileContext(nc) as tc:
        with (
            tc.tile_pool(name="wpool", bufs=1) as wpool,
            tc.tile_pool(name="cpool", bufs=1) as cpool,
            tc.tile_pool(name="xpool", bufs=1) as xpool,
            tc.tile_pool(name="hpool", bufs=1) as hpool,
            tc.tile_pool(name="qpool", bufs=1) as qpool,
            tc.tile_pool(name="kpool", bufs=1) as kpool,
            tc.tile_pool(name="vpool", bufs=1) as vpool,
            tc.tile_pool(name="epool", bufs=1) as epool,
            tc.tile_pool(name="opool", bufs=1) as opool,
            tc.tile_pool(name="gnpool", bufs=2) as gnpool,
            tc.tile_pool(name="spool", bufs=2) as spool,
            tc.tile_pool(name="ps", bufs=8, space="PSUM") as ps,
        ):
            # ---- weights / constants (loaded once) ----
            qkvw_sb = wpool.tile([128, KT, 3 * C], F32R)
            for kt in range(KT):
                nc.sync.dma_start(
                    out=qkvw_sb[:, kt, :],
                    in_=qkvwT_ext.ap()[kt * 128 : (kt + 1) * 128, :],
                )
            projw_sb = wpool.tile([128, KT, C], F32R)
            for kt in range(KT):
                nc.sync.dma_start(
                    out=projw_sb[:, kt, :],
                    in_=projwT_ext.ap()[kt * 128 : (kt + 1) * 128, :],
                )
            # qkv bias as per-o-tile partition columns (128, 12)
            qkvb_col = cpool.tile([128, 12], F32)
            nc.sync.dma_start(
                out=qkvb_col, in_=qkvb_ext.ap().rearrange("(t p) -> p t", p=128)
            )
            # v bias replicated across partitions (for the vT drain add)
            vb_bc = cpool.tile([128, C], F32)
            nc.sync.dma_start(
                out=vb_bc, in_=qkvb_ext.ap()[2 * C : 3 * C].partition_broadcast(128)
            )
            projb_row = cpool.tile([1, C], F32R)
            nc.sync.dma_start(out=projb_row, in_=projb_ext.ap().unsqueeze(0))
            nw_sb = cpool.tile([128, KT], F32)
            nc.sync.dma_start(
                out=nw_sb, in_=nw_ext.ap().rearrange("(t p) -> p t", p=128)
            )
            nb_sb = cpool.tile([128, KT], F32)
            nc.sync.dma_start(
                out=nb_sb, in_=nb_ext.ap().rearrange("(t p) -> p t", p=128)
            )
            ind_sb = cpool.tile([128, 8], F32R)
            nc.sync.dma_start(out=ind_sb, in_=ind_ext.ap())
            indT_sb = cpool.tile([8, 128], F32R)
            nc.sync.dma_start(out=indT_sb, in_=indT_ext.ap())
            ones_row = cpool.tile([1, N], F32R)
            nc.sync.dma_start(out=ones_row, in_=ones_ext.ap().unsqueeze(0))
            ones_col = cpool.tile([128, 1], F32R)
            nc.sync.dma_start(out=ones_col, in_=ones_ext.ap()[0:128].unsqueeze(1))
            eps_sb = cpool.tile([8, 1], F32)
            nc.vector.memset(eps_sb, EPS)

            for s in range(SPC):
                # ---- load x ----
                x_sb = xpool.tile([128, KT, N], F32, tag="x")
                for kt in range(KT):
                    nc.sync.dma_start(
                        out=x_sb[:, kt, :],
                        in_=x_ext.ap()[s, kt * 128 : (kt + 1) * 128, :],
                    )

                # ---- groupnorm stats ----
                stats = gnpool.tile([128, KT, 2, 6], F32, tag="stats")
                for kt in range(KT):
                    for sg in range(2):
                        nc.vector.bn_stats(
                            out=stats[:, kt, sg, :],
                            in_=x_sb[:, kt, sg * 512 : (sg + 1) * 512],
                        )
                mv = gnpool.tile([128, KT, 2], F32, tag="mv")
                for kt in range(KT):
                    nc.vector.bn_aggr(out=mv[:, kt, :], in_=stats[:, kt, :, :])
                # stat2 = [mean, var + mean^2] per partition
                stat2 = gnpool.tile([128, KT, 2], F32R, tag="stat2")
                msq_t = gnpool.tile([128, KT], F32, tag="msqt")
                nc.vector.tensor_mul(msq_t, mv[:, :, 0], mv[:, :, 0])
                nc.vector.tensor_add(stat2[:, :, 1], msq_t, mv[:, :, 1])
                nc.vector.tensor_copy(stat2[:, :, 0], mv[:, :, 0])
                # per-group sums via indicator matmul -> (8 groups, kt, 2)
                ps_gs = ps.tile([8, KT, 2], F32, tag="mm")
                for kt in range(KT):
                    nc.tensor.matmul(
                        ps_gs[:, kt, :],
                        ind_sb,
                        stat2[:, kt, :],
                        start=True,
                        stop=True,
                    )
                # group mean / E[x^2] (divide by 16 partitions per group)
                gs = gnpool.tile([8, KT, 2], F32, tag="gs")
                nc.vector.tensor_scalar_mul(gs, ps_gs, 1.0 / GS)
                msq = gnpool.tile([8, KT], F32, tag="msq")
                nc.vector.tensor_mul(msq, gs[:, :, 0], gs[:, :, 0])
                nc.vector.tensor_sub(gs[:, :, 1], gs[:, :, 1], msq)
                # rstd = 1/sqrt(var + eps), in place in gs[:,:,1]
                nc.scalar.activation(
                    out=gs[:, :, 1], in_=gs[:, :, 1], func=Sqrt, bias=eps_sb
                )
                nc.vector.reciprocal(gs[:, :, 1], gs[:, :, 1])
                # rounded copy [mean, rstd] feeding the broadcast matmul
                mr = gnpool.tile([8, KT, 2], F32R, tag="mr")
                nc.vector.tensor_copy(mr, gs)
                # broadcast group stats to channels, then scale/bias per channel
                scb = gnpool.tile([128, KT, 2], F32, tag="scb")
                h_sb = hpool.tile([128, KT, N], F32R, tag="h")
                for kt in range(KT):
                    ps_bc = ps.tile([128, 2], F32, tag="mm")
                    nc.tensor.matmul(
                        ps_bc, indT_sb, mr[:, kt, :], start=True, stop=True
                    )
                    # scale = rstd * w
                    nc.vector.tensor_mul(
                        scb[:, kt, 0:1], ps_bc[:, 1:2], nw_sb[:, kt : kt + 1]
                    )
                    # bias = b - mean * scale
                    nc.vector.tensor_mul(
                        scb[:, kt, 1:2], ps_bc[:, 0:1], scb[:, kt, 0:1]
                    )
                    nc.vector.tensor_sub(
                        scb[:, kt, 1:2], nb_sb[:, kt : kt + 1], scb[:, kt, 1:2]
                    )
                    # h = x*scale + bias
                    nc.vector.tensor_scalar(
                        out=h_sb[:, kt, :],
                        in0=x_sb[:, kt, :],
                        scalar1=scb[:, kt, 0:1],
                        scalar2=scb[:, kt, 1:2],
                        op0=mult,
                        op1=add,
                    )

                # ---- QKV: q, k in (C, N) layout ----
                q_sb = qpool.tile([128, KT, N], F32R, tag="q")
                k_sb = kpool.tile([128, KT, N], F32R, tag="k")
                for ot in range(8):
                    dest = q_sb if ot < 4 else k_sb
                    oc = ot % 4
                    for ih in range(NH):
                        pm = ps.tile([128, 512], F32, tag="mm")
                        for kt in range(KT):
                            nc.tensor.matmul(
                                pm,
                                qkvw_sb[:, kt, ot * 128 : (ot + 1) * 128],
                                h_sb[:, kt, ih * 512 : (ih + 1) * 512],
                                start=(kt == 0),
                                stop=(kt == KT - 1),
                            )
                        nc.scalar.activation(
                            out=dest[:, oc, ih * 512 : (ih + 1) * 512],
                            in_=pm,
                            func=Identity,
                            bias=qkvb_col[:, ot : ot + 1],
                        )

                # ---- vT = h^T @ v_w^T in (N, C) layout (+ v bias) ----
                vT_sb = vpool.tile([128, NT, C], F32R, tag="vT")
                for nt in range(NT):
                    pm = ps.tile([128, 512], F32, tag="mm")
                    for kt in range(KT):
                        nc.tensor.matmul(
                            pm,
                            h_sb[:, kt, nt * 128 : (nt + 1) * 128],
                            qkvw_sb[:, kt, 2 * C : 3 * C],
                            start=(kt == 0),
                            stop=(kt == KT - 1),
                        )
                    nc.vector.tensor_add(vT_sb[:, nt, :], pm, vb_bc)

                # ---- s = k^T q (keys on partitions); e = exp(s * scale) ----
                e_sb = epool.tile([128, NT, N], F32R, tag="e")
                for jt in range(NT):
                    for ih in range(NH):
                        pm = ps.tile([128, 512], F32, tag="mm")
                        for ot in range(KT):
                            nc.tensor.matmul(
                                pm,
                                k_sb[:, ot, jt * 128 : (jt + 1) * 128],
                                q_sb[:, ot, ih * 512 : (ih + 1) * 512],
                                start=(ot == 0),
                                stop=(ot == KT - 1),
                            )
                        nc.scalar.activation(
                            out=e_sb[:, jt, ih * 512 : (ih + 1) * 512],
                            in_=pm,
                            func=Exp,
                            scale=SCALE,
                        )

                # ---- softmax denominators S = sum_j e ----
                recipS = spool.tile([1, N], F32R, tag="recipS")
                for ih in range(NH):
                    pS = ps.tile([1, 512], F32, tag="mm")
                    for jt in range(NT):
                        nc.tensor.matmul(
                            pS,
                            ones_col,
                            e_sb[:, jt, ih * 512 : (ih + 1) * 512],
                            start=(jt == 0),
                            stop=(jt == NT - 1),
                        )
                    with nc.allow_low_precision(reason="fp32r rounding for PE"):
                        nc.vector.reciprocal(
                            recipS[:, ih * 512 : (ih + 1) * 512], pS
                        )
                # broadcast 1/S across partitions via K=1 matmul
                rSbc = spool.tile([128, N], F32, tag="rSbc")
                for ih in range(NH):
                    pb = ps.tile([128, 512], F32, tag="mm")
                    nc.tensor.matmul(
                        pb,
                        ones_row[:, 0:128],
                        recipS[:, ih * 512 : (ih + 1) * 512],
                        start=True,
                        stop=True,
                    )
                    nc.vector.tensor_copy(rSbc[:, ih * 512 : (ih + 1) * 512], pb)

                # ---- o = vT^T @ e, normalized by 1/S ----
                o_sb = opool.tile([128, KT, N], F32R, tag="o")
                for ct in range(KT):
                    for ih in range(NH):
                        pm = ps.tile([128, 512], F32, tag="mm")
                        for jt in range(NT):
                            nc.tensor.matmul(
                                pm,
                                vT_sb[:, jt, ct * 128 : (ct + 1) * 128],
                                e_sb[:, jt, ih * 512 : (ih + 1) * 512],
                                start=(jt == 0),
                                stop=(jt == NT - 1),
                            )
                        nc.vector.tensor_mul(
                            o_sb[:, ct, ih * 512 : (ih + 1) * 512],
                            pm,
                            rSbc[:, ih * 512 : (ih + 1) * 512],
                        )

                # ---- proj (+bias via K=1 matmul) + residual ----
                y_sb = hpool.tile([128, KT, N], F32, tag="h")  # reuse h slot
                for ct2 in range(KT):
                    for ih in range(NH):
                        pm = ps.tile([128, 512], F32, tag="mm")
                        for ckt in range(KT):
                            nc.tensor.matmul(
                                pm,
                                projw_sb[:, ckt, ct2 * 128 : (ct2 + 1) * 128],
                                o_sb[:, ckt, ih * 512 : (ih + 1) * 512],
                                start=(ckt == 0),
                                stop=False,
                            )
                        nc.tensor.matmul(
                            pm,
                            projb_row[:, ct2 * 128 : (ct2 + 1) * 128],
                            ones_row[:, 0:512],
                            start=False,
                            stop=True,
                        )
                        nc.vector.tensor_add(
                            y_sb[:, ct2, ih * 512 : (ih + 1) * 512],
                            pm,
                            x_sb[:, ct2, ih * 512 : (ih + 1) * 512],
                        )
                    nc.sync.dma_start(
                        out=y_ext.ap()[s, ct2 * 128 : (ct2 + 1) * 128, :],
                        in_=y_sb[:, ct2, :],
                    )

    nc.compile()
    return nc


def _get_nc():
    if "nc" not in _BUILD_CACHE:
        _BUILD_CACHE["nc"] = _build()
    return _BUILD_CACHE["nc"]


def kernel(x, norm_w, norm_b, qkv_w, qkv_b, proj_w, proj_b, _trace=False):
    global LAST_RESULT
    nc = _get_nc()

    x = np.asarray(x, dtype=np.float32).reshape(B, C, N)
    qkvwT = np.ascontiguousarray(np.asarray(qkv_w, dtype=np.float32).T)
    projwT = np.ascontiguousarray(np.asarray(proj_w, dtype=np.float32).T)
    ind16 = np.zeros((128, 8), dtype=np.float32)
    for p in range(128):
        ind16[p, p // GS] = 1.0
    ind16T = np.ascontiguousarray(ind16.T)

    shared = {
        "norm_w": np.ascontiguousarray(norm_w, dtype=np.float32),
        "norm_b": np.ascontiguousarray(norm_b, dtype=np.float32),
        "qkvwT": qkvwT,
        "qkv_b": np.ascontiguousarray(qkv_b, dtype=np.float32),
        "projwT": projwT,
        "proj_b": np.ascontiguousarray(proj_b, dtype=np.float32),
        "ind16": ind16,
        "ind16T": ind16T,
        "ones": np.ones(N, dtype=np.float32),
    }
    in_maps = [
        {"x": np.ascontiguousarray(x[c * SPC : (c + 1) * SPC]), **shared}
        for c in range(NCORES)
    ]
    res = run_bass_kernel_spmd(nc, in_maps, list(range(NCORES)), trace=_trace)
    LAST_RESULT = res
    out = np.concatenate([res.results[i]["y"] for i in range(NCORES)], axis=0)
    return out.reshape(B, C, H, W)


# revision 14
# speedup vs baseline: 1.3169x; 1.3169x over previous
"""AttnBlock (GroupNorm + 1x1-conv QKV + single-head spatial attention + proj
+ residual) on 8 Trainium2 NeuronCores.

Sharding: pure data-parallel over batch — 16 samples / 8 cores = 2 samples per
core; weights broadcast. No collectives; gather on host.

Per-core kernel formulation (per sample, C=512 channels, N=1024 spatial):
  h   = groupnorm(x)                (stats via PE indicator matmuls)
  vT  = h^T @ v_w^T                 (spatial on partitions, C free)
  q,k = qkv_w[:1024] @ h            (C on partitions, spatial free)
  s   = k^T q                       (keys j on partitions, queries i free)
  e   = exp(s * C^-0.5)             (logits are O(1); no max-subtraction needed)
  S   = ones^T e                    (softmax denominators via PE reduction)
  o   = vT^T e * (1/S)              (1/S broadcast across partitions via DRAM bounce)
  y   = x + proj_w @ o + proj_b
All matmuls run as float32r (fp32 storage, single-pass PE mode). The two
samples' phases are interleaved in emission order so the second sample's
GroupNorm (DVE) hides under the first sample's attention matmuls (PE).
"""

import numpy as np

import concourse.bass as bass
import concourse.tile as tile
from concourse import bacc, mybir
from concourse.bass_utils import run_bass_kernel_spmd

B, C, H, W = 16, 512, 32, 32
N = H * W              # 1024 spatial positions
G = 32                 # groups
GS = C // G            # 16 channels per group
NCORES = 8
SPC = B // NCORES      # samples per core
EPS = 1e-6
SCALE = float(C) ** -0.5
KT = C // 128          # 4 channel tiles of 128
NT = N // 128          # 8 spatial tiles of 128
NH = N // 512          # 2 free-dim halves of 512

F32 = mybir.dt.float32
F32R = mybir.dt.float32r

_BUILD_CACHE = {}
LAST_RESULT = None  # BassKernelResults of the most recent run (for test harness)


def _build():
    nc = bacc.Bacc("TRN2", target_bir_lowering=False, debug=False)

    x_ext = nc.declare_dram_parameter("x", [SPC, C, N], F32, isOutput=False)
    nw_ext = nc.declare_dram_parameter("norm_w", [C], F32, isOutput=False)
    nb_ext = nc.declare_dram_parameter("norm_b", [C], F32, isOutput=False)
    qkvwT_ext = nc.declare_dram_parameter("qkvwT", [C, 3 * C], F32R, isOutput=False)
    qkvb_ext = nc.declare_dram_parameter("qkv_b", [3 * C], F32, isOutput=False)
    projwT_ext = nc.declare_dram_parameter("projwT", [C, C], F32R, isOutput=False)
    projb_ext = nc.declare_dram_parameter("proj_b", [C], F32, isOutput=False)
    ind_ext = nc.declare_dram_parameter("ind16", [128, 8], F32R, isOutput=False)
    indT_ext = nc.declare_dram_parameter("ind16T", [8, 128], F32R, isOutput=False)
    ones_ext = nc.declare_dram_parameter("ones", [128], F32R, isOutput=False)
    y_ext = nc.declare_dram_parameter("y", [SPC, C, N], F32, isOutput=True)

    sdram = nc.dram_tensor("rs_bounce", [SPC, N], F32)

    Identity = mybir.ActivationFunctionType.Identity
    Exp = mybir.ActivationFunctionType.Exp
    Sqrt = mybir.ActivationFunctionType.Sqrt
    mult = mybir.AluOpType.mult
    add = mybir.AluOpType.add

    with tile.TileContext(nc) as tc:
        with (
            tc.tile_pool(name="wpool", bufs=1) as wpool,
            tc.tile_pool(name="cpool", bufs=1) as cpool,
            tc.tile_pool(name="xpool", bufs=2) as xpool,
            tc.tile_pool(name="hpool", bufs=1) as hpool,
            tc.tile_pool(name="qpool", bufs=1) as qpool,
            tc.tile_pool(name="kpool", bufs=1) as kpool,
            tc.tile_pool(name="vpool", bufs=1) as vpool,
            tc.tile_pool(name="epool", bufs=1) as epool,
            tc.tile_pool(name="opool", bufs=1) as opool,
            tc.tile_pool(name="gnpool", bufs=2) as gnpool,
            tc.tile_pool(name="spool", bufs=1) as spool,
            tc.tile_pool(name="ps", bufs=8, space="PSUM") as ps,
        ):
            # ---- input x for both samples first (prefetch) ----
            x_tiles = []
            for s in range(SPC):
                x_sb = xpool.tile([128, KT, N], F32, tag="x")
                for kt in range(KT):
                    nc.sync.dma_start(
                        out=x_sb[:, kt, :],
                        in_=x_ext.ap()[s, kt * 128 : (kt + 1) * 128, :],
                    )
                x_tiles.append(x_sb)

            # ---- small constants ----
            qkvb_col = cpool.tile([128, 12], F32)
            nc.sync.dma_start(
                out=qkvb_col, in_=qkvb_ext.ap().rearrange("(t p) -> p t", p=128)
            )
            vb_bc = cpool.tile([128, C], F32)
            nc.sync.dma_start(
                out=vb_bc, in_=qkvb_ext.ap()[2 * C : 3 * C].partition_broadcast(128)
            )
            pb_col = cpool.tile([128, KT], F32)
            nc.sync.dma_start(
                out=pb_col, in_=projb_ext.ap().rearrange("(t p) -> p t", p=128)
            )
            nw_sb = cpool.tile([128, KT], F32)
            nc.sync.dma_start(
                out=nw_sb, in_=nw_ext.ap().rearrange("(t p) -> p t", p=128)
            )
            nb_sb = cpool.tile([128, KT], F32)
            nc.sync.dma_start(
                out=nb_sb, in_=nb_ext.ap().rearrange("(t p) -> p t", p=128)
            )
            ind_sb = cpool.tile([128, 8], F32R)
            nc.sync.dma_start(out=ind_sb, in_=ind_ext.ap())
            indT_sb = cpool.tile([8, 128], F32R)
            nc.sync.dma_start(out=indT_sb, in_=indT_ext.ap())
            ones_col = cpool.tile([128, 1], F32R)
            nc.sync.dma_start(out=ones_col, in_=ones_ext.ap().unsqueeze(1))
            eps_sb = cpool.tile([8, 1], F32)
            nc.vector.memset(eps_sb, EPS)

            # ---- weights (arrive while GroupNorm runs) ----
            qkvw_sb = wpool.tile([128, KT, 3 * C], F32R)
            for kt in range(KT):
                nc.sync.dma_start(
                    out=qkvw_sb[:, kt, :],
                    in_=qkvwT_ext.ap()[kt * 128 : (kt + 1) * 128, :],
                )
            projw_sb = wpool.tile([128, KT, C], F32R)
            for kt in range(KT):
                nc.sync.dma_start(
                    out=projw_sb[:, kt, :],
                    in_=projwT_ext.ap()[kt * 128 : (kt + 1) * 128, :],
                )

            def groupnorm(s):
                """Full GroupNorm for sample s -> returns h tile (F32R)."""
                x_sb = x_tiles[s]
                stats = gnpool.tile([128, KT, 2, 6], F32, tag="stats")
                for kt in range(KT):
                    for sg in range(2):
                        nc.vector.bn_stats(
                            out=stats[:, kt, sg, :],
                            in_=x_sb[:, kt, sg * 512 : (sg + 1) * 512],
                        )
                mv = gnpool.tile([128, KT, 2], F32, tag="mv")
                for kt in range(KT):
                    nc.vector.bn_aggr(out=mv[:, kt, :], in_=stats[:, kt, :, :])
                # stat2 = [mean, var + mean^2] per partition
                stat2 = gnpool.tile([128, KT, 2], F32R, tag="stat2")
                msq_t = gnpool.tile([128, KT], F32, tag="msqt")
                nc.vector.tensor_mul(msq_t, mv[:, :, 0], mv[:, :, 0])
                nc.vector.tensor_add(stat2[:, :, 1], msq_t, mv[:, :, 1])
                nc.vector.tensor_copy(stat2[:, :, 0], mv[:, :, 0])
                # per-group sums via indicator matmul -> (8 groups, kt, 2)
                ps_gs = ps.tile([8, KT, 2], F32, tag="mm")
                for kt in range(KT):
                    nc.tensor.matmul(
                        ps_gs[:, kt, :], ind_sb, stat2[:, kt, :],
                        start=True, stop=True,
                    )
                # group mean / E[x^2] (divide by 16 partitions per group)
                gs = gnpool.tile([8, KT, 2], F32, tag="gs")
                nc.vector.tensor_scalar_mul(gs, ps_gs, 1.0 / GS)
                msq = gnpool.tile([8, KT], F32, tag="msq")
                nc.vector.tensor_mul(msq, gs[:, :, 0], gs[:, :, 0])
                nc.vector.tensor_sub(gs[:, :, 1], gs[:, :, 1], msq)
                nc.scalar.activation(
                    out=gs[:, :, 1], in_=gs[:, :, 1], func=Sqrt, bias=eps_sb
                )
                nc.vector.reciprocal(gs[:, :, 1], gs[:, :, 1])
                # rounded copy [mean, rstd] feeding the broadcast matmul
                mr = gnpool.tile([8, KT, 2], F32R, tag="mr")
                nc.vector.tensor_copy(mr, gs)
                scb = gnpool.tile([128, KT, 2], F32, tag="scb")
                h_sb = hpool.tile([128, KT, N], F32R, tag="h")
                for kt in range(KT):
                    ps_bc = ps.tile([128, 2], F32, tag="mm")
                    nc.tensor.matmul(
                        ps_bc, indT_sb, mr[:, kt, :], start=True, stop=True
                    )
                    nc.vector.tensor_mul(
                        scb[:, kt, 0:1], ps_bc[:, 1:2], nw_sb[:, kt : kt + 1]
                    )
                    nc.vector.tensor_mul(
                        scb[:, kt, 1:2], ps_bc[:, 0:1], scb[:, kt, 0:1]
                    )
                    nc.vector.tensor_sub(
                        scb[:, kt, 1:2], nb_sb[:, kt : kt + 1], scb[:, kt, 1:2]
                    )
                    nc.vector.tensor_scalar(
                        out=h_sb[:, kt, :],
                        in0=x_sb[:, kt, :],
                        scalar1=scb[:, kt, 0:1],
                        scalar2=scb[:, kt, 1:2],
                        op0=mult,
                        op1=add,
                    )
                return h_sb

            def v_transposed(h_sb):
                """vT = h^T @ v_w^T (+ v bias broadcast along free dim)."""
                vT_sb = vpool.tile([128, NT, C], F32R, tag="vT")
                for nt in range(NT):
                    pm = ps.tile([128, 512], F32, tag="mm")
                    for kt in range(KT):
                        nc.tensor.matmul(
                            pm,
                            h_sb[:, kt, nt * 128 : (nt + 1) * 128],
                            qkvw_sb[:, kt, 2 * C : 3 * C],
                            start=(kt == 0),
                            stop=(kt == KT - 1),
                        )
                    nc.vector.tensor_add(vT_sb[:, nt, :], pm, vb_bc)
                return vT_sb

            def qk(h_sb):
                q_sb = qpool.tile([128, KT, N], F32R, tag="q")
                k_sb = kpool.tile([128, KT, N], F32R, tag="k")
                for ot in range(8):
                    dest = q_sb if ot < 4 else k_sb
                    oc = ot % 4
                    for ih in range(NH):
                        pm = ps.tile([128, 512], F32, tag="mm")
                        for kt in range(KT):
                            nc.tensor.matmul(
                                pm,
                                qkvw_sb[:, kt, ot * 128 : (ot + 1) * 128],
                                h_sb[:, kt, ih * 512 : (ih + 1) * 512],
                                start=(kt == 0),
                                stop=(kt == KT - 1),
                            )
                        nc.scalar.activation(
                            out=dest[:, oc, ih * 512 : (ih + 1) * 512],
                            in_=pm,
                            func=Identity,
                            bias=qkvb_col[:, ot : ot + 1],
                        )
                return q_sb, k_sb

            def attention(s, q_sb, k_sb, vT_sb):
                # s = k^T q (keys on partitions); e = exp(s * scale)
                e_sb = epool.tile([128, NT, N], F32R, tag="e")
                for jt in range(NT):
                    for ih in range(NH):
                        pm = ps.tile([128, 512], F32, tag="mm")
                        for ot in range(KT):
                            nc.tensor.matmul(
                                pm,
                                k_sb[:, ot, jt * 128 : (jt + 1) * 128],
                                q_sb[:, ot, ih * 512 : (ih + 1) * 512],
                                start=(ot == 0),
                                stop=(ot == KT - 1),
                            )
                        nc.scalar.activation(
                            out=e_sb[:, jt, ih * 512 : (ih + 1) * 512],
                            in_=pm,
                            func=Exp,
                            scale=SCALE,
                        )
                # softmax denominators S = sum_j e; 1/S broadcast via DRAM
                recipS = spool.tile([1, N], F32, tag="recipS")
                for ih in range(NH):
                    pS = ps.tile([1, 512], F32, tag="mm")
                    for jt in range(NT):
                        nc.tensor.matmul(
                            pS,
                            ones_col,
                            e_sb[:, jt, ih * 512 : (ih + 1) * 512],
                            start=(jt == 0),
                            stop=(jt == NT - 1),
                        )
                    nc.vector.reciprocal_approx_fast(
                        out=recipS[:, ih * 512 : (ih + 1) * 512], in_=pS
                    )
                nc.sync.dma_start(out=sdram.ap()[s].unsqueeze(0), in_=recipS)
                rSbc = spool.tile([128, N], F32, tag="rSbc")
                nc.sync.dma_start(
                    out=rSbc, in_=sdram.ap()[s].partition_broadcast(128)
                )
                # o = vT^T @ e, normalized by 1/S
                o_sb = opool.tile([128, KT, N], F32R, tag="o")
                for ct in range(KT):
                    for ih in range(NH):
                        pm = ps.tile([128, 512], F32, tag="mm")
                        for jt in range(NT):
                            nc.tensor.matmul(
                                pm,
                                vT_sb[:, jt, ct * 128 : (ct + 1) * 128],
                                e_sb[:, jt, ih * 512 : (ih + 1) * 512],
                                start=(jt == 0),
                                stop=(jt == NT - 1),
                            )
                        nc.vector.tensor_mul(
                            o_sb[:, ct, ih * 512 : (ih + 1) * 512],
                            pm,
                            rSbc[:, ih * 512 : (ih + 1) * 512],
                        )
                return o_sb

            def proj_resid(s, o_sb, x_sb):
                # residual accumulates in place into the (now dead) x tile
                for ct2 in range(KT):
                    for ih in range(NH):
                        pm = ps.tile([128, 512], F32, tag="mm")
                        for ckt in range(KT):
                            nc.tensor.matmul(
                                pm,
                                projw_sb[:, ckt, ct2 * 128 : (ct2 + 1) * 128],
                                o_sb[:, ckt, ih * 512 : (ih + 1) * 512],
                                start=(ckt == 0),
                                stop=(ckt == KT - 1),
                            )
                        # + proj bias, in place on PSUM (ScalarE)
                        nc.scalar.activation(
                            out=pm, in_=pm, func=Identity,
                            bias=pb_col[:, ct2 : ct2 + 1],
                        )
                        # + residual, in place into x
                        nc.vector.tensor_add(
                            x_sb[:, ct2, ih * 512 : (ih + 1) * 512],
                            pm,
                            x_sb[:, ct2, ih * 512 : (ih + 1) * 512],
                        )
                    nc.gpsimd.dma_start(
                        out=y_ext.ap()[s, ct2 * 128 : (ct2 + 1) * 128, :],
                        in_=x_sb[:, ct2, :],
                    )

            # ---- interleaved two-sample schedule ----
            h0 = groupnorm(0)
            vT0 = v_transposed(h0)
            q0, k0 = qk(h0)
            h1 = groupnorm(1)          # DVE work hides under sample-0 attention
            o0 = attention(0, q0, k0, vT0)
            proj_resid(0, o0, x_tiles[0])
            vT1 = v_transposed(h1)
            q1, k1 = qk(h1)
            o1 = attention(1, q1, k1, vT1)
            proj_resid(1, o1, x_tiles[1])

    nc.compile()
    return nc


def _get_nc():
    if "nc" not in _BUILD_CACHE:
        _BUILD_CACHE["nc"] = _build()
    return _BUILD_CACHE["nc"]


def kernel(x, norm_w, norm_b, qkv_w, qkv_b, proj_w, proj_b, _trace=False):
    global LAST_RESULT
    nc = _get_nc()

    x = np.asarray(x, dtype=np.float32).reshape(B, C, N)
    qkvwT = np.ascontiguousarray(np.asarray(qkv_w, dtype=np.float32).T)
    projwT = np.ascontiguousarray(np.asarray(proj_w, dtype=np.float32).T)
    ind16 = np.zeros((128, 8), dtype=np.float32)
    for p in range(128):
        ind16[p, p // GS] = 1.0
    ind16T = np.ascontiguousarray(ind16.T)

    shared = {
        "norm_w": np.ascontiguousarray(norm_w, dtype=np.float32),
        "norm_b": np.ascontiguousarray(norm_b, dtype=np.float32),
        "qkvwT": qkvwT,
        "qkv_b": np.ascontiguousarray(qkv_b, dtype=np.float32),
        "projwT": projwT,
        "proj_b": np.ascontiguousarray(proj_b, dtype=np.float32),
        "ind16": ind16,
        "ind16T": ind16T,
        "ones": np.ones(128, dtype=np.float32),
    }
    in_maps = [
        {"x": np.ascontiguousarray(x[c * SPC : (c + 1) * SPC]), **shared}
        for c in range(NCORES)
    ]
    res = run_bass_kernel_spmd(nc, in_maps, list(range(NCORES)), trace=_trace)
    LAST_RESULT = res
    out = np.concatenate([res.results[i]["y"] for i in range(NCORES)], axis=0)
    return out.reshape(B, C, H, W)


# revision 17
# speedup vs baseline: 1.3339x; 1.0129x over previous
"""AttnBlock (GroupNorm + 1x1-conv QKV + single-head spatial attention + proj
+ residual) on 8 Trainium2 NeuronCores.

Sharding: pure data-parallel over batch — 16 samples / 8 cores = 2 samples per
core; weights broadcast. No collectives; gather on host.

Per-core kernel formulation (per sample, C=512 channels, N=1024 spatial):
  h   = groupnorm(x)                (stats via PE indicator matmuls)
  vT  = h^T @ v_w^T                 (spatial on partitions, C free)
  q,k = qkv_w[:1024] @ h            (C on partitions, spatial free)
  s   = k^T q                       (keys j on partitions, queries i free)
  e   = exp(s * C^-0.5)             (logits are O(1); no max-subtraction needed)
  S   = ones^T e                    (softmax denominators via PE reduction)
  o   = vT^T e * (1/S)              (1/S broadcast across partitions via DRAM bounce)
  y   = x + proj_w @ o + proj_b
All matmuls run as float32r (fp32 storage, single-pass PE mode). The two
samples' phases are interleaved in emission order so the second sample's
GroupNorm (DVE) hides under the first sample's attention matmuls (PE).
"""

import numpy as np

import concourse.bass as bass
import concourse.tile as tile
from concourse import bacc, mybir
from concourse.bass_utils import run_bass_kernel_spmd

B, C, H, W = 16, 512, 32, 32
N = H * W              # 1024 spatial positions
G = 32                 # groups
GS = C // G            # 16 channels per group
NCORES = 8
SPC = B // NCORES      # samples per core
EPS = 1e-6
SCALE = float(C) ** -0.5
KT = C // 128          # 4 channel tiles of 128
NT = N // 128          # 8 spatial tiles of 128
NH = N // 512          # 2 free-dim halves of 512

F32 = mybir.dt.float32
F32R = mybir.dt.float32r

_BUILD_CACHE = {}
LAST_RESULT = None  # BassKernelResults of the most recent run (for test harness)


def _build():
    nc = bacc.Bacc("TRN2", target_bir_lowering=False, debug=False)

    x_ext = nc.declare_dram_parameter("x", [SPC, C, N], F32, isOutput=False)
    nw_ext = nc.declare_dram_parameter("norm_w", [C], F32, isOutput=False)
    nb_ext = nc.declare_dram_parameter("norm_b", [C], F32, isOutput=False)
    qkvwT_ext = nc.declare_dram_parameter("qkvwT", [C, 3 * C], F32R, isOutput=False)
    qkvb_ext = nc.declare_dram_parameter("qkv_b", [3 * C], F32, isOutput=False)
    projwT_ext = nc.declare_dram_parameter("projwT", [C, C], F32R, isOutput=False)
    projb_ext = nc.declare_dram_parameter("proj_b", [C], F32, isOutput=False)
    ind_ext = nc.declare_dram_parameter("ind16", [128, 8], F32R, isOutput=False)
    indT_ext = nc.declare_dram_parameter("ind16T", [8, 128], F32R, isOutput=False)
    ones_ext = nc.declare_dram_parameter("ones", [128], F32R, isOutput=False)
    y_ext = nc.declare_dram_parameter("y", [SPC, C, N], F32, isOutput=True)

    sdram = nc.dram_tensor("rs_bounce", [SPC, N], F32)

    Identity = mybir.ActivationFunctionType.Identity
    Exp = mybir.ActivationFunctionType.Exp
    Sqrt = mybir.ActivationFunctionType.Sqrt
    mult = mybir.AluOpType.mult
    add = mybir.AluOpType.add

    with tile.TileContext(nc) as tc:
        with (
            tc.tile_pool(name="wpool", bufs=1) as wpool,
            tc.tile_pool(name="cpool", bufs=1) as cpool,
            tc.tile_pool(name="xpool", bufs=2) as xpool,
            tc.tile_pool(name="hpool", bufs=1) as hpool,
            tc.tile_pool(name="qpool", bufs=1) as qpool,
            tc.tile_pool(name="kpool", bufs=1) as kpool,
            tc.tile_pool(name="vpool", bufs=1) as vpool,
            tc.tile_pool(name="epool", bufs=1) as epool,
            tc.tile_pool(name="opool", bufs=1) as opool,
            tc.tile_pool(name="gnpool", bufs=2) as gnpool,
            tc.tile_pool(name="spool", bufs=1) as spool,
            tc.tile_pool(name="ps", bufs=8, space="PSUM") as ps,
        ):
            # ---- small constants ----
            qkvb_col = cpool.tile([128, 12], F32)
            nc.sync.dma_start(
                out=qkvb_col, in_=qkvb_ext.ap().rearrange("(t p) -> p t", p=128)
            )
            vb_bc = cpool.tile([128, C], F32)
            nc.sync.dma_start(
                out=vb_bc, in_=qkvb_ext.ap()[2 * C : 3 * C].partition_broadcast(128)
            )
            pb_col = cpool.tile([128, KT], F32)
            nc.sync.dma_start(
                out=pb_col, in_=projb_ext.ap().rearrange("(t p) -> p t", p=128)
            )
            nw_sb = cpool.tile([128, KT], F32)
            nc.sync.dma_start(
                out=nw_sb, in_=nw_ext.ap().rearrange("(t p) -> p t", p=128)
            )
            nb_sb = cpool.tile([128, KT], F32)
            nc.sync.dma_start(
                out=nb_sb, in_=nb_ext.ap().rearrange("(t p) -> p t", p=128)
            )
            ind_sb = cpool.tile([128, 8], F32R)
            nc.sync.dma_start(out=ind_sb, in_=ind_ext.ap())
            indT_sb = cpool.tile([8, 128], F32R)
            nc.sync.dma_start(out=indT_sb, in_=indT_ext.ap())
            ones_col = cpool.tile([128, 1], F32R)
            nc.sync.dma_start(out=ones_col, in_=ones_ext.ap().unsqueeze(1))
            eps_sb = cpool.tile([8, 1], F32)
            nc.vector.memset(eps_sb, EPS)

            # ---- inputs/weights in consumption order: x0, qkv weights, x1, proj ----
            x_tiles = []
            for s in range(SPC):
                x_tiles.append(
                    xpool.tile([128, KT, N], F32, tag="x", name=f"x_sb{s}")
                )
            qkvw_sb = wpool.tile([128, KT, 3 * C], F32R)
            projw_sb = wpool.tile([128, KT, C], F32R)
            for kt in range(KT):
                nc.sync.dma_start(
                    out=x_tiles[0][:, kt, :],
                    in_=x_ext.ap()[0, kt * 128 : (kt + 1) * 128, :],
                )
            for kt in range(KT):
                nc.sync.dma_start(
                    out=qkvw_sb[:, kt, :],
                    in_=qkvwT_ext.ap()[kt * 128 : (kt + 1) * 128, :],
                )
            for kt in range(KT):
                nc.sync.dma_start(
                    out=x_tiles[1][:, kt, :],
                    in_=x_ext.ap()[1, kt * 128 : (kt + 1) * 128, :],
                )
            for kt in range(KT):
                nc.sync.dma_start(
                    out=projw_sb[:, kt, :],
                    in_=projwT_ext.ap()[kt * 128 : (kt + 1) * 128, :],
                )

            def groupnorm(s):
                """Full GroupNorm for sample s -> returns h tile (F32R)."""
                x_sb = x_tiles[s]
                stats = gnpool.tile([128, KT, 2, 6], F32, tag="stats")
                for kt in range(KT):
                    for sg in range(2):
                        nc.vector.bn_stats(
                            out=stats[:, kt, sg, :],
                            in_=x_sb[:, kt, sg * 512 : (sg + 1) * 512],
                        )
                mv = gnpool.tile([128, KT, 2], F32, tag="mv")
                for kt in range(KT):
                    nc.vector.bn_aggr(out=mv[:, kt, :], in_=stats[:, kt, :, :])
                # stat2 = [mean, var + mean^2] per partition
                stat2 = gnpool.tile([128, KT, 2], F32R, tag="stat2")
                msq_t = gnpool.tile([128, KT], F32, tag="msqt")
                nc.vector.tensor_mul(msq_t, mv[:, :, 0], mv[:, :, 0])
                nc.vector.tensor_add(stat2[:, :, 1], msq_t, mv[:, :, 1])
                nc.vector.tensor_copy(stat2[:, :, 0], mv[:, :, 0])
                # per-group sums via indicator matmul -> (8 groups, kt, 2)
                ps_gs = ps.tile([8, KT, 2], F32, tag="mm")
                for kt in range(KT):
                    nc.tensor.matmul(
                        ps_gs[:, kt, :], ind_sb, stat2[:, kt, :],
                        start=True, stop=True,
                    )
                # group mean / E[x^2] (divide by 16 partitions per group)
                gs = gnpool.tile([8, KT, 2], F32, tag="gs")
                nc.vector.tensor_scalar_mul(gs, ps_gs, 1.0 / GS)
                msq = gnpool.tile([8, KT], F32, tag="msq")
                nc.vector.tensor_mul(msq, gs[:, :, 0], gs[:, :, 0])
                nc.vector.tensor_sub(gs[:, :, 1], gs[:, :, 1], msq)
                nc.scalar.activation(
                    out=gs[:, :, 1], in_=gs[:, :, 1], func=Sqrt, bias=eps_sb
                )
                nc.vector.reciprocal(gs[:, :, 1], gs[:, :, 1])
                # rounded copy [mean, rstd] feeding the broadcast matmul
                mr = gnpool.tile([8, KT, 2], F32R, tag="mr")
                nc.vector.tensor_copy(mr, gs)
                scb = gnpool.tile([128, KT, 2], F32, tag="scb")
                h_sb = hpool.tile([128, KT, N], F32R, tag="h")
                for kt in range(KT):
                    ps_bc = ps.tile([128, 2], F32, tag="mm")
                    nc.tensor.matmul(
                        ps_bc, indT_sb, mr[:, kt, :], start=True, stop=True
                    )
                    nc.vector.tensor_mul(
                        scb[:, kt, 0:1], ps_bc[:, 1:2], nw_sb[:, kt : kt + 1]
                    )
                    nc.vector.tensor_mul(
                        scb[:, kt, 1:2], ps_bc[:, 0:1], scb[:, kt, 0:1]
                    )
                    nc.vector.tensor_sub(
                        scb[:, kt, 1:2], nb_sb[:, kt : kt + 1], scb[:, kt, 1:2]
                    )
                    nc.vector.tensor_scalar(
                        out=h_sb[:, kt, :],
                        in0=x_sb[:, kt, :],
                        scalar1=scb[:, kt, 0:1],
                        scalar2=scb[:, kt, 1:2],
                        op0=mult,
                        op1=add,
                    )
                return h_sb

            def v_transposed(h_sb):
                """vT = h^T @ v_w^T (+ v bias broadcast along free dim)."""
                vT_sb = vpool.tile([128, NT, C], F32R, tag="vT")
                for nt in range(NT):
                    pm = ps.tile([128, 512], F32, tag="mm")
                    for kt in range(KT):
                        nc.tensor.matmul(
                            pm,
                            h_sb[:, kt, nt * 128 : (nt + 1) * 128],
                            qkvw_sb[:, kt, 2 * C : 3 * C],
                            start=(kt == 0),
                            stop=(kt == KT - 1),
                        )
                    nc.vector.tensor_add(vT_sb[:, nt, :], pm, vb_bc)
                return vT_sb

            def qk(h_sb):
                q_sb = qpool.tile([128, KT, N], F32R, tag="q")
                k_sb = kpool.tile([128, KT, N], F32R, tag="k")
                for ot in range(8):
                    dest = q_sb if ot < 4 else k_sb
                    oc = ot % 4
                    for ih in range(NH):
                        pm = ps.tile([128, 512], F32, tag="mm")
                        for kt in range(KT):
                            nc.tensor.matmul(
                                pm,
                                qkvw_sb[:, kt, ot * 128 : (ot + 1) * 128],
                                h_sb[:, kt, ih * 512 : (ih + 1) * 512],
                                start=(kt == 0),
                                stop=(kt == KT - 1),
                            )
                        nc.scalar.activation(
                            out=dest[:, oc, ih * 512 : (ih + 1) * 512],
                            in_=pm,
                            func=Identity,
                            bias=qkvb_col[:, ot : ot + 1],
                        )
                return q_sb, k_sb

            def attention(s, q_sb, k_sb, vT_sb):
                # s = k^T q (keys on partitions); e = exp(s * scale)
                e_sb = epool.tile([128, NT, N], F32R, tag="e")
                for jt in range(NT):
                    for ih in range(NH):
                        pm = ps.tile([128, 512], F32, tag="mm")
                        for ot in range(KT):
                            nc.tensor.matmul(
                                pm,
                                k_sb[:, ot, jt * 128 : (jt + 1) * 128],
                                q_sb[:, ot, ih * 512 : (ih + 1) * 512],
                                start=(ot == 0),
                                stop=(ot == KT - 1),
                            )
                        nc.scalar.activation(
                            out=e_sb[:, jt, ih * 512 : (ih + 1) * 512],
                            in_=pm,
                            func=Exp,
                            scale=SCALE,
                        )
                # softmax denominators S = sum_j e; 1/S broadcast via DRAM
                recipS = spool.tile([1, N], F32, tag="recipS")
                for ih in range(NH):
                    pS = ps.tile([1, 512], F32, tag="mm")
                    for jt in range(NT):
                        nc.tensor.matmul(
                            pS,
                            ones_col,
                            e_sb[:, jt, ih * 512 : (ih + 1) * 512],
                            start=(jt == 0),
                            stop=(jt == NT - 1),
                        )
                    nc.vector.reciprocal_approx_fast(
                        out=recipS[:, ih * 512 : (ih + 1) * 512], in_=pS
                    )
                nc.sync.dma_start(out=sdram.ap()[s].unsqueeze(0), in_=recipS)
                rSbc = spool.tile([128, N], F32, tag="rSbc")
                nc.sync.dma_start(
                    out=rSbc, in_=sdram.ap()[s].partition_broadcast(128)
                )
                # o = vT^T @ e, normalized by 1/S
                o_sb = opool.tile([128, KT, N], F32R, tag="o")
                for ct in range(KT):
                    for ih in range(NH):
                        pm = ps.tile([128, 512], F32, tag="mm")
                        for jt in range(NT):
                            nc.tensor.matmul(
                                pm,
                                vT_sb[:, jt, ct * 128 : (ct + 1) * 128],
                                e_sb[:, jt, ih * 512 : (ih + 1) * 512],
                                start=(jt == 0),
                                stop=(jt == NT - 1),
                            )
                        nc.vector.tensor_mul(
                            o_sb[:, ct, ih * 512 : (ih + 1) * 512],
                            pm,
                            rSbc[:, ih * 512 : (ih + 1) * 512],
                        )
                return o_sb

            def proj_resid(s, o_sb, x_sb):
                # residual accumulates in place into the (now dead) x tile
                for ct2 in range(KT):
                    for ih in range(NH):
                        pm = ps.tile([128, 512], F32, tag="mm")
                        for ckt in range(KT):
                            nc.tensor.matmul(
                                pm,
                                projw_sb[:, ckt, ct2 * 128 : (ct2 + 1) * 128],
                                o_sb[:, ckt, ih * 512 : (ih + 1) * 512],
                                start=(ckt == 0),
                                stop=(ckt == KT - 1),
                            )
                        # + proj bias, in place on PSUM (ScalarE)
                        nc.scalar.activation(
                            out=pm, in_=pm, func=Identity,
                            bias=pb_col[:, ct2 : ct2 + 1],
                        )
                        # + residual, in place into x
                        nc.vector.tensor_add(
                            x_sb[:, ct2, ih * 512 : (ih + 1) * 512],
                            pm,
                            x_sb[:, ct2, ih * 512 : (ih + 1) * 512],
                        )
                        nc.gpsimd.dma_start(
                            out=y_ext.ap()[
                                s,
                                ct2 * 128 : (ct2 + 1) * 128,
                                ih * 512 : (ih + 1) * 512,
                            ],
                            in_=x_sb[:, ct2, ih * 512 : (ih + 1) * 512],
                        )

            # ---- interleaved two-sample schedule ----
            h0 = groupnorm(0)
            vT0 = v_transposed(h0)
            q0, k0 = qk(h0)
            h1 = groupnorm(1)          # DVE work hides under sample-0 attention
            o0 = attention(0, q0, k0, vT0)
            proj_resid(0, o0, x_tiles[0])
            vT1 = v_transposed(h1)
            q1, k1 = qk(h1)
            o1 = attention(1, q1, k1, vT1)
            proj_resid(1, o1, x_tiles[1])

    nc.compile()
    return nc


def _get_nc():
    if "nc" not in _BUILD_CACHE:
        _BUILD_CACHE["nc"] = _build()
    return _BUILD_CACHE["nc"]


def kernel(x, norm_w, norm_b, qkv_w, qkv_b, proj_w, proj_b, _trace=False):
    global LAST_RESULT
    nc = _get_nc()

    x = np.asarray(x, dtype=np.float32).reshape(B, C, N)
    qkvwT = np.ascontiguousarray(np.asarray(qkv_w, dtype=np.float32).T)
    projwT = np.ascontiguousarray(np.asarray(proj_w, dtype=np.float32).T)
    ind16 = np.zeros((128, 8), dtype=np.float32)
    for p in range(128):
        ind16[p, p // GS] = 1.0
    ind16T = np.ascontiguousarray(ind16.T)

    shared = {
        "norm_w": np.ascontiguousarray(norm_w, dtype=np.float32),
        "norm_b": np.ascontiguousarray(norm_b, dtype=np.float32),
        "qkvwT": qkvwT,
        "qkv_b": np.ascontiguousarray(qkv_b, dtype=np.float32),
        "projwT": projwT,
        "proj_b": np.ascontiguousarray(proj_b, dtype=np.float32),
        "ind16": ind16,
        "ind16T": ind16T,
        "ones": np.ones(128, dtype=np.float32),
    }
    in_maps = [
        {"x": np.ascontiguousarray(x[c * SPC : (c + 1) * SPC]), **shared}
        for c in range(NCORES)
    ]
    res = run_bass_kernel_spmd(nc, in_maps, list(range(NCORES)), trace=_trace)
    LAST_RESULT = res
    out = np.concatenate([res.results[i]["y"] for i in range(NCORES)], axis=0)
    return out.reshape(B, C, H, W)


# revision 19
# speedup vs baseline: 1.3573x; 1.0175x over previous
"""AttnBlock (GroupNorm + 1x1-conv QKV + single-head spatial attention + proj
+ residual) on 8 Trainium2 NeuronCores.

Sharding: pure data-parallel over batch — 16 samples / 8 cores = 2 samples per
core; weights broadcast. No collectives; gather on host.

Per-core kernel formulation (per sample, C=512 channels, N=1024 spatial):
  h   = groupnorm(x)                (stats via PE indicator matmuls)
  vT  = h^T @ v_w^T                 (spatial on partitions, C free)
  q,k = qkv_w[:1024] @ h            (C on partitions, spatial free)
  s   = k^T q                       (keys j on partitions, queries i free)
  e   = exp(s * C^-0.5)             (logits are O(1); no max-subtraction needed)
  S   = ones^T e                    (softmax denominators via PE reduction)
  o   = vT^T e * (1/S)              (1/S broadcast across partitions via DRAM bounce)
  y   = x + proj_w @ o + proj_b
All matmuls run as float32r (fp32 storage, single-pass PE mode). The two
samples' phases are interleaved in emission order so the second sample's
GroupNorm (DVE) hides under the first sample's attention matmuls (PE).
"""

import numpy as np

import concourse.bass as bass
import concourse.tile as tile
from concourse import bacc, mybir
from concourse.bass_utils import run_bass_kernel_spmd

B, C, H, W = 16, 512, 32, 32
N = H * W              # 1024 spatial positions
G = 32                 # groups
GS = C // G            # 16 channels per group
NCORES = 8
SPC = B // NCORES      # samples per core
EPS = 1e-6
SCALE = float(C) ** -0.5
KT = C // 128          # 4 channel tiles of 128
NT = N // 128          # 8 spatial tiles of 128
NH = N // 512          # 2 free-dim halves of 512

F32 = mybir.dt.float32
F32R = mybir.dt.float32r

_BUILD_CACHE = {}
LAST_RESULT = None  # BassKernelResults of the most recent run (for test harness)


def _build():
    nc = bacc.Bacc("TRN2", target_bir_lowering=False, debug=False)

    x_ext = nc.declare_dram_parameter("x", [SPC, C, N], F32, isOutput=False)
    qkvwT_ext = nc.declare_dram_parameter("qkvwT", [C, 3 * C], F32R, isOutput=False)
    projwT_ext = nc.declare_dram_parameter("projwT", [C, C], F32R, isOutput=False)
    qkvbc_ext = nc.declare_dram_parameter("qkvb_col", [128, 12], F32, isOutput=False)
    cst_ext = nc.declare_dram_parameter("consts12", [128, 12], F32, isOutput=False)
    vbbc_ext = nc.declare_dram_parameter("vb_bc", [128, C], F32, isOutput=False)
    ind_ext = nc.declare_dram_parameter("ind16", [128, 8], F32R, isOutput=False)
    indT_ext = nc.declare_dram_parameter("ind16T", [8, 128], F32R, isOutput=False)
    ones_ext = nc.declare_dram_parameter("ones", [128], F32R, isOutput=False)
    y_ext = nc.declare_dram_parameter("y", [SPC, C, N], F32, isOutput=True)

    sdram = nc.dram_tensor("rs_bounce", [SPC, N], F32)

    Identity = mybir.ActivationFunctionType.Identity
    Exp = mybir.ActivationFunctionType.Exp
    Sqrt = mybir.ActivationFunctionType.Sqrt
    mult = mybir.AluOpType.mult
    add = mybir.AluOpType.add

    with tile.TileContext(nc) as tc:
        with (
            tc.tile_pool(name="wpool", bufs=1) as wpool,
            tc.tile_pool(name="cpool", bufs=1) as cpool,
            tc.tile_pool(name="xpool", bufs=2) as xpool,
            tc.tile_pool(name="hpool", bufs=1) as hpool,
            tc.tile_pool(name="qpool", bufs=1) as qpool,
            tc.tile_pool(name="kpool", bufs=1) as kpool,
            tc.tile_pool(name="vpool", bufs=1) as vpool,
            tc.tile_pool(name="epool", bufs=1) as epool,
            tc.tile_pool(name="opool", bufs=1) as opool,
            tc.tile_pool(name="gnpool", bufs=2) as gnpool,
            tc.tile_pool(name="spool", bufs=1) as spool,
            tc.tile_pool(name="ps", bufs=8, space="PSUM") as ps,
        ):
            # ---- x sample 0 first (256KB chunks across queues) ----
            x_tiles = []
            for s in range(SPC):
                x_tiles.append(
                    xpool.tile([128, KT, N], F32, tag="x", name=f"x_sb{s}")
                )
            for kt in range(KT):
                for sg in range(2):
                    nc.sync.dma_start(
                        out=x_tiles[0][:, kt, sg * 512 : (sg + 1) * 512],
                        in_=x_ext.ap()[
                            0, kt * 128 : (kt + 1) * 128, sg * 512 : (sg + 1) * 512
                        ],
                    )

            # ---- small constants (host-pre-shaped, contiguous loads) ----
            qkvb_col = cpool.tile([128, 12], F32)
            nc.sync.dma_start(out=qkvb_col, in_=qkvbc_ext.ap())
            cst_sb = cpool.tile([128, 12], F32)
            nc.sync.dma_start(out=cst_sb, in_=cst_ext.ap())
            nw_sb = cst_sb[:, 0:4]
            nb_sb = cst_sb[:, 4:8]
            pb_col = cst_sb[:, 8:12]
            vb_bc = cpool.tile([128, C], F32)
            nc.sync.dma_start(out=vb_bc, in_=vbbc_ext.ap())
            ind_sb = cpool.tile([128, 8], F32R)
            nc.sync.dma_start(out=ind_sb, in_=ind_ext.ap())
            indT_sb = cpool.tile([8, 128], F32R)
            nc.sync.dma_start(out=indT_sb, in_=indT_ext.ap())
            ones_col = cpool.tile([128, 1], F32R)
            nc.sync.dma_start(out=ones_col, in_=ones_ext.ap().unsqueeze(1))
            eps_sb = cpool.tile([8, 1], F32)
            nc.vector.memset(eps_sb, EPS)

            # ---- weights (v columns first per kt), then x1, then proj ----
            qkvw_sb = wpool.tile([128, KT, 3 * C], F32R)
            projw_sb = wpool.tile([128, KT, C], F32R)
            for kt in range(KT):
                for chunk in (2, 0, 1):  # v, q, k column blocks
                    nc.sync.dma_start(
                        out=qkvw_sb[:, kt, chunk * C : (chunk + 1) * C],
                        in_=qkvwT_ext.ap()[
                            kt * 128 : (kt + 1) * 128, chunk * C : (chunk + 1) * C
                        ],
                    )
            for kt in range(KT):
                for sg in range(2):
                    nc.sync.dma_start(
                        out=x_tiles[1][:, kt, sg * 512 : (sg + 1) * 512],
                        in_=x_ext.ap()[
                            1, kt * 128 : (kt + 1) * 128, sg * 512 : (sg + 1) * 512
                        ],
                    )
            for kt in range(KT):
                nc.sync.dma_start(
                    out=projw_sb[:, kt, :],
                    in_=projwT_ext.ap()[kt * 128 : (kt + 1) * 128, :],
                )

            def groupnorm(s):
                """Full GroupNorm for sample s -> returns h tile (F32R)."""
                x_sb = x_tiles[s]
                stats = gnpool.tile([128, KT, 2, 6], F32, tag="stats")
                for kt in range(KT):
                    for sg in range(2):
                        nc.vector.bn_stats(
                            out=stats[:, kt, sg, :],
                            in_=x_sb[:, kt, sg * 512 : (sg + 1) * 512],
                        )
                mv = gnpool.tile([128, KT, 2], F32, tag="mv")
                for kt in range(KT):
                    nc.vector.bn_aggr(out=mv[:, kt, :], in_=stats[:, kt, :, :])
                # stat2 = [mean, var + mean^2] per partition
                stat2 = gnpool.tile([128, KT, 2], F32R, tag="stat2")
                msq_t = gnpool.tile([128, KT], F32, tag="msqt")
                nc.vector.tensor_mul(msq_t, mv[:, :, 0], mv[:, :, 0])
                nc.vector.tensor_add(stat2[:, :, 1], msq_t, mv[:, :, 1])
                nc.vector.tensor_copy(stat2[:, :, 0], mv[:, :, 0])
                # per-group sums via indicator matmul -> (8 groups, kt, 2)
                ps_gs = ps.tile([8, KT, 2], F32, tag="mm")
                for kt in range(KT):
                    nc.tensor.matmul(
                        ps_gs[:, kt, :], ind_sb, stat2[:, kt, :],
                        start=True, stop=True,
                    )
                # group mean / E[x^2] (divide by 16 partitions per group)
                gs = gnpool.tile([8, KT, 2], F32, tag="gs")
                nc.vector.tensor_scalar_mul(gs, ps_gs, 1.0 / GS)
                msq = gnpool.tile([8, KT], F32, tag="msq")
                nc.vector.tensor_mul(msq, gs[:, :, 0], gs[:, :, 0])
                nc.vector.tensor_sub(gs[:, :, 1], gs[:, :, 1], msq)
                nc.scalar.activation(
                    out=gs[:, :, 1], in_=gs[:, :, 1], func=Sqrt, bias=eps_sb
                )
                nc.vector.reciprocal(gs[:, :, 1], gs[:, :, 1])
                # rounded copy [mean, rstd] feeding the broadcast matmul
                mr = gnpool.tile([8, KT, 2], F32R, tag="mr")
                nc.vector.tensor_copy(mr, gs)
                scb = gnpool.tile([128, KT, 2], F32, tag="scb")
                h_sb = hpool.tile([128, KT, N], F32R, tag="h")
                for kt in range(KT):
                    ps_bc = ps.tile([128, 2], F32, tag="mm")
                    nc.tensor.matmul(
                        ps_bc, indT_sb, mr[:, kt, :], start=True, stop=True
                    )
                    nc.vector.tensor_mul(
                        scb[:, kt, 0:1], ps_bc[:, 1:2], nw_sb[:, kt : kt + 1]
                    )
                    nc.vector.tensor_mul(
                        scb[:, kt, 1:2], ps_bc[:, 0:1], scb[:, kt, 0:1]
                    )
                    nc.vector.tensor_sub(
                        scb[:, kt, 1:2], nb_sb[:, kt : kt + 1], scb[:, kt, 1:2]
                    )
                    nc.vector.tensor_scalar(
                        out=h_sb[:, kt, :],
                        in0=x_sb[:, kt, :],
                        scalar1=scb[:, kt, 0:1],
                        scalar2=scb[:, kt, 1:2],
                        op0=mult,
                        op1=add,
                    )
                return h_sb

            def v_transposed(h_sb):
                """vT = h^T @ v_w^T (+ v bias broadcast along free dim)."""
                vT_sb = vpool.tile([128, NT, C], F32R, tag="vT")
                for nt in range(NT):
                    pm = ps.tile([128, 512], F32, tag="mm")
                    for kt in range(KT):
                        nc.tensor.matmul(
                            pm,
                            h_sb[:, kt, nt * 128 : (nt + 1) * 128],
                            qkvw_sb[:, kt, 2 * C : 3 * C],
                            start=(kt == 0),
                            stop=(kt == KT - 1),
                        )
                    nc.vector.tensor_add(vT_sb[:, nt, :], pm, vb_bc)
                return vT_sb

            def qk(h_sb):
                q_sb = qpool.tile([128, KT, N], F32R, tag="q")
                k_sb = kpool.tile([128, KT, N], F32R, tag="k")
                for ot in range(8):
                    dest = q_sb if ot < 4 else k_sb
                    oc = ot % 4
                    for ih in range(NH):
                        pm = ps.tile([128, 512], F32, tag="mm")
                        for kt in range(KT):
                            nc.tensor.matmul(
                                pm,
                                qkvw_sb[:, kt, ot * 128 : (ot + 1) * 128],
                                h_sb[:, kt, ih * 512 : (ih + 1) * 512],
                                start=(kt == 0),
                                stop=(kt == KT - 1),
                            )
                        nc.scalar.activation(
                            out=dest[:, oc, ih * 512 : (ih + 1) * 512],
                            in_=pm,
                            func=Identity,
                            bias=qkvb_col[:, ot : ot + 1],
                        )
                return q_sb, k_sb

            def attention(s, q_sb, k_sb, vT_sb):
                # s = k^T q (keys on partitions); e = exp(s * scale)
                e_sb = epool.tile([128, NT, N], F32R, tag="e")
                for jt in range(NT):
                    for ih in range(NH):
                        pm = ps.tile([128, 512], F32, tag="mm")
                        for ot in range(KT):
                            nc.tensor.matmul(
                                pm,
                                k_sb[:, ot, jt * 128 : (jt + 1) * 128],
                                q_sb[:, ot, ih * 512 : (ih + 1) * 512],
                                start=(ot == 0),
                                stop=(ot == KT - 1),
                            )
                        nc.scalar.activation(
                            out=e_sb[:, jt, ih * 512 : (ih + 1) * 512],
                            in_=pm,
                            func=Exp,
                            scale=SCALE,
                        )
                # softmax denominators S = sum_j e; 1/S broadcast via DRAM
                recipS = spool.tile([1, N], F32, tag="recipS")
                for ih in range(NH):
                    pS = ps.tile([1, 512], F32, tag="mm")
                    for jt in range(NT):
                        nc.tensor.matmul(
                            pS,
                            ones_col,
                            e_sb[:, jt, ih * 512 : (ih + 1) * 512],
                            start=(jt == 0),
                            stop=(jt == NT - 1),
                        )
                    nc.vector.reciprocal_approx_fast(
                        out=recipS[:, ih * 512 : (ih + 1) * 512], in_=pS
                    )
                nc.sync.dma_start(out=sdram.ap()[s].unsqueeze(0), in_=recipS)
                rSbc = spool.tile([128, N], F32, tag="rSbc")
                nc.sync.dma_start(
                    out=rSbc, in_=sdram.ap()[s].partition_broadcast(128)
                )
                # o = vT^T @ e, normalized by 1/S
                o_sb = opool.tile([128, KT, N], F32R, tag="o")
                for ct in range(KT):
                    for ih in range(NH):
                        pm = ps.tile([128, 512], F32, tag="mm")
                        for jt in range(NT):
                            nc.tensor.matmul(
                                pm,
                                vT_sb[:, jt, ct * 128 : (ct + 1) * 128],
                                e_sb[:, jt, ih * 512 : (ih + 1) * 512],
                                start=(jt == 0),
                                stop=(jt == NT - 1),
                            )
                        nc.vector.tensor_mul(
                            o_sb[:, ct, ih * 512 : (ih + 1) * 512],
                            pm,
                            rSbc[:, ih * 512 : (ih + 1) * 512],
                        )
                return o_sb

            def proj_resid(s, o_sb, x_sb):
                # residual accumulates in place into the (now dead) x tile
                for ct2 in range(KT):
                    for ih in range(NH):
                        pm = ps.tile([128, 512], F32, tag="mm")
                        for ckt in range(KT):
                            nc.tensor.matmul(
                                pm,
                                projw_sb[:, ckt, ct2 * 128 : (ct2 + 1) * 128],
                                o_sb[:, ckt, ih * 512 : (ih + 1) * 512],
                                start=(ckt == 0),
                                stop=(ckt == KT - 1),
                            )
                        # + proj bias, in place on PSUM (ScalarE)
                        nc.scalar.activation(
                            out=pm, in_=pm, func=Identity,
                            bias=pb_col[:, ct2 : ct2 + 1],
                        )
                        # + residual, in place into x
                        nc.vector.tensor_add(
                            x_sb[:, ct2, ih * 512 : (ih + 1) * 512],
                            pm,
                            x_sb[:, ct2, ih * 512 : (ih + 1) * 512],
                        )
                        nc.gpsimd.dma_start(
                            out=y_ext.ap()[
                                s,
                                ct2 * 128 : (ct2 + 1) * 128,
                                ih * 512 : (ih + 1) * 512,
                            ],
                            in_=x_sb[:, ct2, ih * 512 : (ih + 1) * 512],
                        )

            # ---- interleaved two-sample schedule ----
            h0 = groupnorm(0)
            vT0 = v_transposed(h0)
            q0, k0 = qk(h0)
            h1 = groupnorm(1)          # DVE work hides under sample-0 attention
            o0 = attention(0, q0, k0, vT0)
            proj_resid(0, o0, x_tiles[0])
            vT1 = v_transposed(h1)
            q1, k1 = qk(h1)
            o1 = attention(1, q1, k1, vT1)
            proj_resid(1, o1, x_tiles[1])

    nc.compile()
    return nc


def _get_nc():
    if "nc" not in _BUILD_CACHE:
        _BUILD_CACHE["nc"] = _build()
    return _BUILD_CACHE["nc"]


def kernel(x, norm_w, norm_b, qkv_w, qkv_b, proj_w, proj_b, _trace=False):
    global LAST_RESULT
    nc = _get_nc()

    x = np.asarray(x, dtype=np.float32).reshape(B, C, N)
    qkvwT = np.ascontiguousarray(np.asarray(qkv_w, dtype=np.float32).T)
    projwT = np.ascontiguousarray(np.asarray(proj_w, dtype=np.float32).T)
    ind16 = np.zeros((128, 8), dtype=np.float32)
    for p in range(128):
        ind16[p, p // GS] = 1.0
    ind16T = np.ascontiguousarray(ind16.T)

    norm_w = np.asarray(norm_w, dtype=np.float32)
    norm_b = np.asarray(norm_b, dtype=np.float32)
    qkv_b = np.asarray(qkv_b, dtype=np.float32)
    proj_b = np.asarray(proj_b, dtype=np.float32)
    # per-o-tile bias columns: col t holds bias[t*128 : (t+1)*128]
    qkvb_col = np.ascontiguousarray(qkv_b.reshape(12, 128).T)
    consts12 = np.ascontiguousarray(
        np.concatenate(
            [
                norm_w.reshape(KT, 128).T,
                norm_b.reshape(KT, 128).T,
                proj_b.reshape(KT, 128).T,
            ],
            axis=1,
        )
    )
    vb_bc = np.ascontiguousarray(
        np.broadcast_to(qkv_b[2 * C : 3 * C], (128, C))
    )
    shared = {
        "qkvwT": qkvwT,
        "projwT": projwT,
        "qkvb_col": qkvb_col,
        "consts12": consts12,
        "vb_bc": vb_bc,
        "ind16": ind16,
        "ind16T": ind16T,
        "ones": np.ones(128, dtype=np.float32),
    }
    in_maps = [
        {"x": np.ascontiguousarray(x[c * SPC : (c + 1) * SPC]), **shared}
        for c in range(NCORES)
    ]
    res = run_bass_kernel_spmd(nc, in_maps, list(range(NCORES)), trace=_trace)
    LAST_RESULT = res
    out = np.concatenate([res.results[i]["y"] for i in range(NCORES)], axis=0)
    return out.reshape(B, C, H, W)


# revision 20
# speedup vs baseline: 1.3596x; 1.0017x over previous
"""AttnBlock (GroupNorm + 1x1-conv QKV + single-head spatial attention + proj
+ residual) on 8 Trainium2 NeuronCores.

Sharding: pure data-parallel over batch — 16 samples / 8 cores = 2 samples per
core; weights broadcast. No collectives; gather on host.

Per-core kernel formulation (per sample, C=512 channels, N=1024 spatial):
  h   = groupnorm(x)                (stats via PE indicator matmuls)
  vT  = h^T @ v_w^T                 (spatial on partitions, C free)
  q,k = qkv_w[:1024] @ h            (C on partitions, spatial free)
  s   = k^T q                       (keys j on partitions, queries i free)
  e   = exp(s * C^-0.5)             (logits are O(1); no max-subtraction needed)
  S   = ones^T e                    (softmax denominators via PE reduction)
  o   = vT^T e * (1/S)              (1/S broadcast across partitions via DRAM bounce)
  y   = x + proj_w @ o + proj_b
All matmuls run as float32r (fp32 storage, single-pass PE mode). The two
samples' phases are interleaved in emission order so the second sample's
GroupNorm (DVE) hides under the first sample's attention matmuls (PE).
"""

import numpy as np

import concourse.bass as bass
import concourse.tile as tile
from concourse import bacc, mybir
from concourse.bass_utils import run_bass_kernel_spmd

B, C, H, W = 16, 512, 32, 32
N = H * W              # 1024 spatial positions
G = 32                 # groups
GS = C // G            # 16 channels per group
NCORES = 8
SPC = B // NCORES      # samples per core
EPS = 1e-6
SCALE = float(C) ** -0.5
KT = C // 128          # 4 channel tiles of 128
NT = N // 128          # 8 spatial tiles of 128
NH = N // 512          # 2 free-dim halves of 512

F32 = mybir.dt.float32
F32R = mybir.dt.float32r

_BUILD_CACHE = {}
LAST_RESULT = None  # BassKernelResults of the most recent run (for test harness)


def _build():
    nc = bacc.Bacc("TRN2", target_bir_lowering=False, debug=False)

    x_ext = nc.declare_dram_parameter("x", [SPC, C, N], F32, isOutput=False)
    qkvwT_ext = nc.declare_dram_parameter("qkvwT", [C, 3 * C], F32R, isOutput=False)
    projwT_ext = nc.declare_dram_parameter("projwT", [C, C], F32R, isOutput=False)
    qkvbc_ext = nc.declare_dram_parameter("qkvb_col", [128, 12], F32, isOutput=False)
    cst_ext = nc.declare_dram_parameter("consts12", [128, 12], F32, isOutput=False)
    vbbc_ext = nc.declare_dram_parameter("vb_bc", [128, C], F32, isOutput=False)
    ind_ext = nc.declare_dram_parameter("ind16", [128, 8], F32R, isOutput=False)
    indT_ext = nc.declare_dram_parameter("ind16T", [8, 128], F32R, isOutput=False)
    ones_ext = nc.declare_dram_parameter("ones", [128], F32R, isOutput=False)
    y_ext = nc.declare_dram_parameter("y", [SPC, C, N], F32, isOutput=True)

    sdram = nc.dram_tensor("rs_bounce", [SPC, N], F32)

    Identity = mybir.ActivationFunctionType.Identity
    Exp = mybir.ActivationFunctionType.Exp
    Sqrt = mybir.ActivationFunctionType.Sqrt
    mult = mybir.AluOpType.mult
    add = mybir.AluOpType.add

    with tile.TileContext(nc) as tc:
        with (
            tc.tile_pool(name="wpool", bufs=1) as wpool,
            tc.tile_pool(name="cpool", bufs=1) as cpool,
            tc.tile_pool(name="xpool", bufs=2) as xpool,
            tc.tile_pool(name="hpool", bufs=1) as hpool,
            tc.tile_pool(name="qpool", bufs=1) as qpool,
            tc.tile_pool(name="kpool", bufs=1) as kpool,
            tc.tile_pool(name="vpool", bufs=1) as vpool,
            tc.tile_pool(name="epool", bufs=1) as epool,
            tc.tile_pool(name="opool", bufs=1) as opool,
            tc.tile_pool(name="gnpool", bufs=2) as gnpool,
            tc.tile_pool(name="spool", bufs=1) as spool,
            tc.tile_pool(name="ps", bufs=8, space="PSUM") as ps,
        ):
            # ---- x sample 0 first (256KB chunks across queues) ----
            x_tiles = []
            for s in range(SPC):
                x_tiles.append(
                    xpool.tile([128, KT, N], F32, tag="x", name=f"x_sb{s}")
                )
            for kt in range(KT):
                for sg in range(2):
                    nc.sync.dma_start(
                        out=x_tiles[0][:, kt, sg * 512 : (sg + 1) * 512],
                        in_=x_ext.ap()[
                            0, kt * 128 : (kt + 1) * 128, sg * 512 : (sg + 1) * 512
                        ],
                    )

            # ---- small constants (host-pre-shaped, contiguous loads) ----
            qkvb_col = cpool.tile([128, 12], F32)
            nc.sync.dma_start(out=qkvb_col, in_=qkvbc_ext.ap())
            cst_sb = cpool.tile([128, 12], F32)
            nc.sync.dma_start(out=cst_sb, in_=cst_ext.ap())
            nw_sb = cst_sb[:, 0:4]
            nb_sb = cst_sb[:, 4:8]
            pb_col = cst_sb[:, 8:12]
            vb_bc = cpool.tile([128, C], F32)
            nc.sync.dma_start(out=vb_bc, in_=vbbc_ext.ap())
            ind_sb = cpool.tile([128, 8], F32R)
            nc.sync.dma_start(out=ind_sb, in_=ind_ext.ap())
            indT_sb = cpool.tile([8, 128], F32R)
            nc.sync.dma_start(out=indT_sb, in_=indT_ext.ap())
            ones_col = cpool.tile([128, 1], F32R)
            nc.sync.dma_start(out=ones_col, in_=ones_ext.ap().unsqueeze(1))
            eps_sb = cpool.tile([8, 1], F32)
            nc.vector.memset(eps_sb, EPS)

            # ---- weights (v columns first per kt), then x1, then proj ----
            qkvw_sb = wpool.tile([128, KT, 3 * C], F32R)
            projw_sb = wpool.tile([128, KT, C], F32R)
            for kt in range(KT):
                for chunk in (2, 0, 1):  # v, q, k column blocks
                    nc.sync.dma_start(
                        out=qkvw_sb[:, kt, chunk * C : (chunk + 1) * C],
                        in_=qkvwT_ext.ap()[
                            kt * 128 : (kt + 1) * 128, chunk * C : (chunk + 1) * C
                        ],
                    )
            for kt in range(KT):
                for sg in range(2):
                    nc.sync.dma_start(
                        out=x_tiles[1][:, kt, sg * 512 : (sg + 1) * 512],
                        in_=x_ext.ap()[
                            1, kt * 128 : (kt + 1) * 128, sg * 512 : (sg + 1) * 512
                        ],
                    )
            for kt in range(KT):
                nc.sync.dma_start(
                    out=projw_sb[:, kt, :],
                    in_=projwT_ext.ap()[kt * 128 : (kt + 1) * 128, :],
                )

            def groupnorm(s):
                """Full GroupNorm for sample s -> returns h tile (F32R)."""
                x_sb = x_tiles[s]
                stats = gnpool.tile([128, KT, 2, 6], F32, tag="stats")
                mv = gnpool.tile([128, KT, 2], F32, tag="mv")
                stat2 = gnpool.tile([128, KT, 2], F32R, tag="stat2")
                msq_t = gnpool.tile([128, KT], F32, tag="msqt")
                ps_gs = ps.tile([8, KT, 2], F32, tag="mm")
                for kt in range(KT):
                    for sg in range(2):
                        nc.vector.bn_stats(
                            out=stats[:, kt, sg, :],
                            in_=x_sb[:, kt, sg * 512 : (sg + 1) * 512],
                        )
                    nc.vector.bn_aggr(out=mv[:, kt, :], in_=stats[:, kt, :, :])
                    # stat2 = [mean, var + mean^2] per partition
                    nc.vector.tensor_mul(
                        msq_t[:, kt : kt + 1], mv[:, kt, 0:1], mv[:, kt, 0:1]
                    )
                    nc.vector.tensor_add(
                        stat2[:, kt, 1:2], msq_t[:, kt : kt + 1], mv[:, kt, 1:2]
                    )
                    nc.vector.tensor_copy(stat2[:, kt, 0:1], mv[:, kt, 0:1])
                    # per-group sums via indicator matmul -> (8 groups, 2)
                    nc.tensor.matmul(
                        ps_gs[:, kt, :], ind_sb, stat2[:, kt, :],
                        start=True, stop=True,
                    )
                # group mean / E[x^2] (divide by 16 partitions per group)
                gs = gnpool.tile([8, KT, 2], F32, tag="gs")
                nc.vector.tensor_scalar_mul(gs, ps_gs, 1.0 / GS)
                msq = gnpool.tile([8, KT], F32, tag="msq")
                nc.vector.tensor_mul(msq, gs[:, :, 0], gs[:, :, 0])
                nc.vector.tensor_sub(gs[:, :, 1], gs[:, :, 1], msq)
                nc.scalar.activation(
                    out=gs[:, :, 1], in_=gs[:, :, 1], func=Sqrt, bias=eps_sb
                )
                nc.vector.reciprocal(gs[:, :, 1], gs[:, :, 1])
                # rounded copy [mean, rstd] feeding the broadcast matmul
                mr = gnpool.tile([8, KT, 2], F32R, tag="mr")
                nc.vector.tensor_copy(mr, gs)
                scb = gnpool.tile([128, KT, 2], F32, tag="scb")
                h_sb = hpool.tile([128, KT, N], F32R, tag="h")
                for kt in range(KT):
                    ps_bc = ps.tile([128, 2], F32, tag="mm")
                    nc.tensor.matmul(
                        ps_bc, indT_sb, mr[:, kt, :], start=True, stop=True
                    )
                    nc.vector.tensor_mul(
                        scb[:, kt, 0:1], ps_bc[:, 1:2], nw_sb[:, kt : kt + 1]
                    )
                    nc.vector.tensor_mul(
                        scb[:, kt, 1:2], ps_bc[:, 0:1], scb[:, kt, 0:1]
                    )
                    nc.vector.tensor_sub(
                        scb[:, kt, 1:2], nb_sb[:, kt : kt + 1], scb[:, kt, 1:2]
                    )
                    nc.vector.tensor_scalar(
                        out=h_sb[:, kt, :],
                        in0=x_sb[:, kt, :],
                        scalar1=scb[:, kt, 0:1],
                        scalar2=scb[:, kt, 1:2],
                        op0=mult,
                        op1=add,
                    )
                return h_sb

            def v_transposed(h_sb):
                """vT = h^T @ v_w^T (+ v bias broadcast along free dim)."""
                vT_sb = vpool.tile([128, NT, C], F32R, tag="vT")
                for nt in range(NT):
                    pm = ps.tile([128, 512], F32, tag="mm")
                    for kt in range(KT):
                        nc.tensor.matmul(
                            pm,
                            h_sb[:, kt, nt * 128 : (nt + 1) * 128],
                            qkvw_sb[:, kt, 2 * C : 3 * C],
                            start=(kt == 0),
                            stop=(kt == KT - 1),
                        )
                    nc.vector.tensor_add(vT_sb[:, nt, :], pm, vb_bc)
                return vT_sb

            def qk(h_sb):
                q_sb = qpool.tile([128, KT, N], F32R, tag="q")
                k_sb = kpool.tile([128, KT, N], F32R, tag="k")
                for ih in range(NH):
                    for ot in range(8):
                        dest = q_sb if ot < 4 else k_sb
                        oc = ot % 4
                        pm = ps.tile([128, 512], F32, tag="mm")
                        for kt in range(KT):
                            nc.tensor.matmul(
                                pm,
                                qkvw_sb[:, kt, ot * 128 : (ot + 1) * 128],
                                h_sb[:, kt, ih * 512 : (ih + 1) * 512],
                                start=(kt == 0),
                                stop=(kt == KT - 1),
                            )
                        nc.scalar.activation(
                            out=dest[:, oc, ih * 512 : (ih + 1) * 512],
                            in_=pm,
                            func=Identity,
                            bias=qkvb_col[:, ot : ot + 1],
                        )
                return q_sb, k_sb

            def attention(s, q_sb, k_sb, vT_sb):
                # s = k^T q (keys on partitions); e = exp(s * scale)
                e_sb = epool.tile([128, NT, N], F32R, tag="e")
                for jt in range(NT):
                    for ih in range(NH):
                        pm = ps.tile([128, 512], F32, tag="mm")
                        for ot in range(KT):
                            nc.tensor.matmul(
                                pm,
                                k_sb[:, ot, jt * 128 : (jt + 1) * 128],
                                q_sb[:, ot, ih * 512 : (ih + 1) * 512],
                                start=(ot == 0),
                                stop=(ot == KT - 1),
                            )
                        nc.scalar.activation(
                            out=e_sb[:, jt, ih * 512 : (ih + 1) * 512],
                            in_=pm,
                            func=Exp,
                            scale=SCALE,
                        )
                # softmax denominators S = sum_j e; 1/S broadcast via DRAM
                recipS = spool.tile([1, N], F32, tag="recipS")
                for ih in range(NH):
                    pS = ps.tile([1, 512], F32, tag="mm")
                    for jt in range(NT):
                        nc.tensor.matmul(
                            pS,
                            ones_col,
                            e_sb[:, jt, ih * 512 : (ih + 1) * 512],
                            start=(jt == 0),
                            stop=(jt == NT - 1),
                        )
                    nc.vector.reciprocal_approx_fast(
                        out=recipS[:, ih * 512 : (ih + 1) * 512], in_=pS
                    )
                nc.sync.dma_start(out=sdram.ap()[s].unsqueeze(0), in_=recipS)
                rSbc = spool.tile([128, N], F32, tag="rSbc")
                nc.sync.dma_start(
                    out=rSbc, in_=sdram.ap()[s].partition_broadcast(128)
                )
                # o = vT^T @ e, normalized by 1/S
                o_sb = opool.tile([128, KT, N], F32R, tag="o")
                for ct in range(KT):
                    for ih in range(NH):
                        pm = ps.tile([128, 512], F32, tag="mm")
                        for jt in range(NT):
                            nc.tensor.matmul(
                                pm,
                                vT_sb[:, jt, ct * 128 : (ct + 1) * 128],
                                e_sb[:, jt, ih * 512 : (ih + 1) * 512],
                                start=(jt == 0),
                                stop=(jt == NT - 1),
                            )
                        nc.vector.tensor_mul(
                            o_sb[:, ct, ih * 512 : (ih + 1) * 512],
                            pm,
                            rSbc[:, ih * 512 : (ih + 1) * 512],
                        )
                return o_sb

            def proj_resid(s, o_sb, x_sb):
                # residual accumulates in place into the (now dead) x tile
                for ct2 in range(KT):
                    for ih in range(NH):
                        pm = ps.tile([128, 512], F32, tag="mm")
                        for ckt in range(KT):
                            nc.tensor.matmul(
                                pm,
                                projw_sb[:, ckt, ct2 * 128 : (ct2 + 1) * 128],
                                o_sb[:, ckt, ih * 512 : (ih + 1) * 512],
                                start=(ckt == 0),
                                stop=(ckt == KT - 1),
                            )
                        # + proj bias, in place on PSUM (ScalarE)
                        nc.scalar.activation(
                            out=pm, in_=pm, func=Identity,
                            bias=pb_col[:, ct2 : ct2 + 1],
                        )
                        # + residual, in place into x
                        nc.vector.tensor_add(
                            x_sb[:, ct2, ih * 512 : (ih + 1) * 512],
                            pm,
                            x_sb[:, ct2, ih * 512 : (ih + 1) * 512],
                        )
                        nc.gpsimd.dma_start(
                            out=y_ext.ap()[
                                s,
                                ct2 * 128 : (ct2 + 1) * 128,
                                ih * 512 : (ih + 1) * 512,
                            ],
                            in_=x_sb[:, ct2, ih * 512 : (ih + 1) * 512],
                        )

            # ---- interleaved two-sample schedule ----
            h0 = groupnorm(0)
            vT0 = v_transposed(h0)
            q0, k0 = qk(h0)
            h1 = groupnorm(1)          # DVE work hides under sample-0 attention
            o0 = attention(0, q0, k0, vT0)
            proj_resid(0, o0, x_tiles[0])
            vT1 = v_transposed(h1)
            q1, k1 = qk(h1)
            o1 = attention(1, q1, k1, vT1)
            proj_resid(1, o1, x_tiles[1])

    nc.compile()
    return nc


def _get_nc():
    if "nc" not in _BUILD_CACHE:
        _BUILD_CACHE["nc"] = _build()
    return _BUILD_CACHE["nc"]


def kernel(x, norm_w, norm_b, qkv_w, qkv_b, proj_w, proj_b, _trace=False):
    global LAST_RESULT
    nc = _get_nc()

    x = np.asarray(x, dtype=np.float32).reshape(B, C, N)
    qkvwT = np.ascontiguousarray(np.asarray(qkv_w, dtype=np.float32).T)
    projwT = np.ascontiguousarray(np.asarray(proj_w, dtype=np.float32).T)
    ind16 = np.zeros((128, 8), dtype=np.float32)
    for p in range(128):
        ind16[p, p // GS] = 1.0
    ind16T = np.ascontiguousarray(ind16.T)

    norm_w = np.asarray(norm_w, dtype=np.float32)
    norm_b = np.asarray(norm_b, dtype=np.float32)
    qkv_b = np.asarray(qkv_b, dtype=np.float32)
    proj_b = np.asarray(proj_b, dtype=np.float32)
    # per-o-tile bias columns: col t holds bias[t*128 : (t+1)*128]
    qkvb_col = np.ascontiguousarray(qkv_b.reshape(12, 128).T)
    consts12 = np.ascontiguousarray(
        np.concatenate(
            [
                norm_w.reshape(KT, 128).T,
                norm_b.reshape(KT, 128).T,
                proj_b.reshape(KT, 128).T,
            ],
            axis=1,
        )
    )
    vb_bc = np.ascontiguousarray(
        np.broadcast_to(qkv_b[2 * C : 3 * C], (128, C))
    )
    shared = {
        "qkvwT": qkvwT,
        "projwT": projwT,
        "qkvb_col": qkvb_col,
        "consts12": consts12,
        "vb_bc": vb_bc,
        "ind16": ind16,
        "ind16T": ind16T,
        "ones": np.ones(128, dtype=np.float32),
    }
    in_maps = [
        {"x": np.ascontiguousarray(x[c * SPC : (c + 1) * SPC]), **shared}
        for c in range(NCORES)
    ]
    res = run_bass_kernel_spmd(nc, in_maps, list(range(NCORES)), trace=_trace)
    LAST_RESULT = res
    out = np.concatenate([res.results[i]["y"] for i in range(NCORES)], axis=0)
    return out.reshape(B, C, H, W)
